# revision 19
# baseline (speedup 1.0000x reference)
"""Trainium2 Bass kernel for nn_ConvDual (quantum-conv hybrid head).

Math: the reference's 4-qubit circuit only entangles qubit pairs (0,1) and
(2,3) (CNOTs are intra-pair), and <Z_0> is invariant under the final CNOT and
the layer-2 RY on wire 1.  It collapses to the closed form

    q_out = A*cos(t0) + B*cos(t0)*cos(t1)
    A = cos(ry[1,0])*cos(ry[0,0]);  B = -sin(ry[1,0])*sin(ry[0,0])*sin(ry[0,1])

with t0 = x[i,j], t1 = x[i,j+1] for the 2x2 patch at (i,j).  The whole model
then reduces, per image, to a cos() map plus shifted-window reductions:

    out[b] = T[b]/16129 + h0*conv_b + head_b
    T[b] = sum_i { (aa+ab)[i]*Rfull[i,b] - ab[i]*x[i,0] - aa[i]*x[i,127]
                   + h1*m[i]*Q[i,b] },   Q[i,b] = sum_j (A + B*C[i,j+1])*C[i,j]
    C = cos(x); Rfull[i,b] = sum_j x[i,b,j]
    aa[i] = h0*(w00*m[i] + w10*m'[i]);  ab[i] = h0*(w01*m[i] + w11*m'[i])
    m[i] = [i<127]; m'[i] = [i>0]

Device strategy (8 images per core, pure data parallel over 8 cores): raw
Bass (no Tile framework — hand-placed semaphores, so the kernel carries none
of Tile's multi-microsecond entry/exit barriers).  The packed input loads as
two partition-halves, one per HWDGE ring (SP + ACT), halving descriptor
generation time.  Compute: two Sin activations write PACKED shifted cos
windows (cos(x)=sin(pi/2-x), argument within the ACT table range), and two
custom-DVE cumulative-sum ops stream

    xcum[i,k] = cumsum_k x[i,k]
    qcum[i,k] = cumsum_k (B*C_next + A)*C_cur

Per-image windowed sums are differences of cumsum samples, which the tensor
engine reads directly as strided views: six K=128 matmuls accumulate
+end/-prev_end samples (and raw boundary columns) against per-row weight
columns into one (1,8) PSUM tile -> SBUF -> DRAM.  Host applies the final
affine.  A 2-op gpsimd tail (dma_reset + sem_clear) restores semaphore state
so the loaded NEFF is safely re-executable.  All runtime scalars ride in 9
weight columns packed ahead of x, keeping the NEFF input-value-independent
(cacheable)."""

import numpy as np

N_CORES = 8
B_FULL, H, W = 64, 128, 128
PB = B_FULL // N_CORES  # images per core
NPATCH = float((H - 1) * (W - 1))
NW = 9  # weight columns packed before x
NXW = NW + PB * W
NQ = PB * (W - 1)

_CACHE = {}


def _get_cumsum_op():
    """Define (once) a custom DVE op: out[k] = cumsum_k (in0*s0 + s1)*in1."""
    if "op" in _CACHE:
        return _CACHE["op"]
    from concourse import dve_ops
    from concourse.dve_spec import Spec, Src0, Src1, C0, C1, Zero, scan, AluOp, lower, _has_src1
    from concourse.dve_uop import DveOpSpec

    NAME = "AFFINE_MUL_CUMSUM_ANT"
    spec = Spec(
        body=scan(AluOp.ADD, (Src0 * C0 + C1) * Src1, init=Zero),
        reference=lambda in0, in1, s0, s1, imm2: np.cumsum(
            (in0.astype(np.float32) * s0 + s1) * in1,
            axis=-1,
            dtype=np.float32,
        ),
    )
    existing = {op.name: op for op in dve_ops.OPS}
    if NAME in existing:
        _CACHE["op"] = existing[NAME]
        return existing[NAME]
    row = max(dve_ops._SUB_OPCODE_FOR_NAME.values()) + 1
    assert row < 0x20
    shas = {}
    for ver in ("v3", "v4"):
        uops = lower(spec, ver=ver)
        shas[ver] = DveOpSpec(
            name=NAME, opcode=row, uops=uops, rd1_en=_has_src1(spec)
        ).sha(ver)
    op = dve_ops.DveOp(NAME, spec, subdim=False, uops_sha=shas)
    dve_ops.OPS.append(op)
    dve_ops._SUB_OPCODE_FOR_NAME[NAME] = row
    dve_ops.CUSTOM_DVE_SPECS[NAME] = spec
    _CACHE["op"] = op
    return op


def _build_program():
    from concourse import bacc
    import concourse.mybir as mybir

    cumsum_op = _get_cumsum_op()

    f32 = mybir.dt.float32
    AF = mybir.ActivationFunctionType

    nc = bacc.Bacc("TRN2", debug=False, num_devices=N_CORES)
    xwd = nc.dram_tensor("xw", [128, NXW], f32, kind="ExternalInput")
    od = nc.dram_tensor("out", [1, PB], f32, kind="ExternalOutput")

    xwt = nc.alloc_sbuf_tensor("xwt", [128, NXW], f32)
    cta = nc.alloc_sbuf_tensor("cta", [128, NQ], f32)  # C[i, b, 0:127] packed
    ctb = nc.alloc_sbuf_tensor("ctb", [128, NQ], f32)  # C[i, b, 1:128] packed
    xcum = nc.alloc_sbuf_tensor("xcum", [128, PB * W], f32)
    qcum = nc.alloc_sbuf_tensor("qcum", [128, NQ], f32)
    ot = nc.alloc_sbuf_tensor("ot", [1, PB], f32)
    pt = nc.alloc_psum_tensor("pt", [1, PB], f32)

    s_dma = nc.alloc_semaphore("s_dma")
    s_sin = nc.alloc_semaphore("s_sin")
    s_dvex = nc.alloc_semaphore("s_dvex")
    s_dveq = nc.alloc_semaphore("s_dveq")
    s_pe = nc.alloc_semaphore("s_pe")
    s_cp = nc.alloc_semaphore("s_cp")
    s_out = nc.alloc_semaphore("s_out")
    sems = [s_dma, s_sin, s_dvex, s_dveq, s_pe, s_cp, s_out]

    xw = xwt.ap()
    wt = xw[:, 0:NW]
    xt = xw[:, NW:NXW]  # partition=i, col=b*W+j
    xv = xt.rearrange("p (b j) -> p b j", b=PB)
    ctav = cta.ap().rearrange("p (b j) -> p b j", j=W - 1)
    ctbv = ctb.ap().rearrange("p (b j) -> p b j", j=W - 1)
    xcv = xcum.ap().rearrange("p (b j) -> p b j", b=PB)
    qcv = qcum.ap().rearrange("p (b j) -> p b j", b=PB)

    with nc.Block(no_gpsimd_drain=True) as block:

        @block.sync
        def _(sp):
            # one full-width DMA: 128 partitions feed all 16 SBUF ports
            sp.dma_start(out=xw[:, :], in_=xwd[:, :]).then_inc(s_dma, 16)
            sp.wait_ge(s_cp, 1)
            sp.dma_start(out=od[:], in_=ot.ap()).then_inc(s_out, 16)
            sp.wait_ge(s_out, 16)

        @block.scalar
        def _(act):
            act.wait_ge(s_dma, 16)
            # C = cos(x) = sin(pi/2 - x); argument in (-pi/2, pi/2]
            act.activation(
                out=ctav, in_=xv[:, :, 0 : W - 1],
                func=AF.Sin, bias=wt[:, 8:9], scale=-1.0,
            ).then_inc(s_sin, 1)
            act.activation(
                out=ctbv, in_=xv[:, :, 1:W],
                func=AF.Sin, bias=wt[:, 8:9], scale=-1.0,
            ).then_inc(s_sin, 1)

        @block.vector
        def _(dve):
            dve.wait_ge(s_dma, 16)
            dve._custom_dve(
                cumsum_op, out=xcum.ap(), in0=xt, in1=xt, s0=0.0, s1=1.0
            ).then_inc(s_dvex, 1)
            dve.wait_ge(s_sin, 2)
            dve._custom_dve(
                cumsum_op, out=qcum.ap(), in0=ctb.ap(), in1=cta.ap(),
                s0=wt[:, 6:7], s1=wt[:, 7:8],
            ).then_inc(s_dveq, 1)
            dve.wait_ge(s_pe, 1)
            dve.tensor_copy(out=ot.ap(), in_=pt.ap()).then_inc(s_cp, 1)

        @block.tensor
        def _(pe):
            p = pt.ap()
            kw = dict(skip_group_check=True)
            pe.wait_ge(s_dvex, 1)  # xcum ready => DMA also observed
            pe.matmul(p[:, 0:PB], wt[:, 0:1], xcv[:, :, W - 1], start=True, stop=False, **kw)
            pe.matmul(p[:, 1:PB], wt[:, 1:2], xcv[:, 0 : PB - 1, W - 1], start=False, stop=False, **kw)
            pe.matmul(p[:, 0:PB], wt[:, 2:3], xv[:, :, 0], start=False, stop=False, **kw)
            pe.matmul(p[:, 0:PB], wt[:, 3:4], xv[:, :, W - 1], start=False, stop=False, **kw)
            pe.wait_ge(s_dveq, 1)
            pe.matmul(p[:, 0:PB], wt[:, 4:5], qcv[:, :, W - 2], start=False, stop=False, **kw)
            pe.matmul(p[:, 1:PB], wt[:, 5:6], qcv[:, 0 : PB - 1, W - 2], start=False, stop=True, **kw).then_inc(s_pe, 1)

        @block.gpsimd
        def _(gp):
            # post-run semaphore/DGE reset so the loaded NEFF can re-execute
            gp.wait_ge(s_out, 16)
            rng = range(min(s.num for s in sems), max(s.num for s in sems) + 1)
            gp.dma_reset(rng)
            gp.sem_clear(rng)

    nc.compile()
    return nc


def _weights(conv_w, conv_b, ry_angles, head_w, head_b):
    ry = np.asarray(ry_angles, np.float64)
    A = np.cos(ry[1, 0]) * np.cos(ry[0, 0])
    Bq = -np.sin(ry[1, 0]) * np.sin(ry[0, 0]) * np.sin(ry[0, 1])
    cw = np.asarray(conv_w, np.float64).reshape(4)
    hw = np.asarray(head_w, np.float64)
    h0, h1 = hw[0, 0], hw[0, 1]
    i = np.arange(128)
    m = (i < H - 1).astype(np.float64)
    mp = (i > 0).astype(np.float64)
    aa = h0 * (cw[0] * m + cw[2] * mp)
    ab = h0 * (cw[1] * m + cw[3] * mp)
    wt = np.zeros((128, NW), np.float32)
    wt[:, 0] = aa + ab
    wt[:, 1] = -(aa + ab)
    wt[:, 2] = -ab
    wt[:, 3] = -aa
    wt[:, 4] = h1 * m
    wt[:, 5] = -(h1 * m)
    wt[:, 6] = Bq
    wt[:, 7] = A
    wt[:, 8] = np.pi / 2  # bias column for cos(x) = sin(-x + pi/2)
    const = h0 * float(np.asarray(conv_b).reshape(-1)[0]) + float(
        np.asarray(head_b).reshape(-1)[0]
    )
    return wt, const


def run(x, conv_w, conv_b, ry_angles, head_w, head_b, trace=False, **run_kwargs):
    from concourse.bass_utils import run_bass_kernel_spmd

    if "nc" not in _CACHE:
        _CACHE["nc"] = _build_program()
    nc = _CACHE["nc"]

    wt, const = _weights(conv_w, conv_b, ry_angles, head_w, head_b)
    xs = np.asarray(x, np.float32).reshape(N_CORES, PB, H, W)
    # per-core packed input: [weight cols | x rows-major-by-partition]
    xperm = xs.transpose(0, 2, 1, 3).reshape(N_CORES, H, PB * W)
    xw = np.empty((N_CORES, H, NXW), np.float32)
    xw[:, :, :NW] = wt[None]
    xw[:, :, NW:] = xperm
    in_maps = [{"xw": xw[c]} for c in range(N_CORES)]
    res = run_bass_kernel_spmd(
        nc, in_maps, list(range(N_CORES)), trace=trace, **run_kwargs
    )
    T = np.concatenate(
        [np.asarray(res.results[c]["out"], np.float64).reshape(PB) for c in range(N_CORES)]
    )
    out = (T / NPATCH + const).astype(np.float32)[:, None]
    return out, res


def kernel(**inputs):
    out, _ = run(**inputs)
    return out


# revision 20
# speedup vs baseline: 1.1181x; 1.1181x over previous
"""Trainium2 Bass kernel for nn_ConvDual (quantum-conv hybrid head).

Math: the reference's 4-qubit circuit only entangles qubit pairs (0,1) and
(2,3) (CNOTs are intra-pair), and <Z_0> is invariant under the final CNOT and
the layer-2 RY on wire 1.  It collapses to the closed form

    q_out = A*cos(t0) + B*cos(t0)*cos(t1)
    A = cos(ry[1,0])*cos(ry[0,0]);  B = -sin(ry[1,0])*sin(ry[0,0])*sin(ry[0,1])

with t0 = x[i,j], t1 = x[i,j+1] for the 2x2 patch at (i,j).  The whole model
then reduces, per image, to a cos() map plus shifted-window reductions:

    out[b] = T[b]/16129 + h0*conv_b + head_b
    T[b] = sum_i { (aa+ab)[i]*Rfull[i,b] - ab[i]*x[i,0] - aa[i]*x[i,127]
                   + h1*m[i]*Q[i,b] },   Q[i,b] = sum_j (A + B*C[i,j+1])*C[i,j]
    C = cos(x); Rfull[i,b] = sum_j x[i,b,j]
    aa[i] = h0*(w00*m[i] + w10*m'[i]);  ab[i] = h0*(w01*m[i] + w11*m'[i])
    m[i] = [i<127]; m'[i] = [i>0]

Device strategy (8 images per core, pure data parallel over 8 cores):
one packed DMA per core, one Sin activation (cos(x)=sin(pi/2-x), argument
within the ACT table range), and TWO custom-DVE cumulative-sum ops:

    xcum[i,k] = cumsum_k x[i,k]                  (over all 8 images' columns)
    qcum[i,k] = cumsum_k (B*C_next + A)*C_cur    (1016 products)

Per-image windowed sums are then differences of cumsum samples, which the
tensor engine reads directly as strided views: six K=128 matmuls accumulate
+end/-prev_end samples (and the raw first/last columns for the conv boundary
correction) against per-row weight columns into a single (1,8) PSUM tile,
which is DMA'd straight to DRAM.  Host applies the final affine.  All
runtime scalars ride in 9 weight columns packed ahead of x in the one input
tensor, so the compiled NEFF is input-value-independent (cacheable).
"""

import numpy as np

N_CORES = 8
B_FULL, H, W = 64, 128, 128
PB = B_FULL // N_CORES  # images per core
NPATCH = float((H - 1) * (W - 1))
NW = 9  # weight columns packed before x
NXW = NW + PB * W

_CACHE = {}


def _get_cumsum_op():
    """Define (once) a custom DVE op: out[k] = cumsum_k (in0*s0 + s1)*in1."""
    if "op" in _CACHE:
        return _CACHE["op"]
    from concourse import dve_ops
    from concourse.dve_spec import Spec, Src0, Src1, C0, C1, Zero, scan, AluOp, lower, _has_src1
    from concourse.dve_uop import DveOpSpec

    NAME = "AFFINE_MUL_CUMSUM_ANT"
    spec = Spec(
        body=scan(AluOp.ADD, (Src0 * C0 + C1) * Src1, init=Zero),
        reference=lambda in0, in1, s0, s1, imm2: np.cumsum(
            (in0.astype(np.float32) * s0 + s1) * in1,
            axis=-1,
            dtype=np.float32,
        ),
    )
    existing = {op.name: op for op in dve_ops.OPS}
    if NAME in existing:
        _CACHE["op"] = existing[NAME]
        return existing[NAME]
    row = max(dve_ops._SUB_OPCODE_FOR_NAME.values()) + 1
    assert row < 0x20
    shas = {}
    for ver in ("v3", "v4"):
        uops = lower(spec, ver=ver)
        shas[ver] = DveOpSpec(
            name=NAME, opcode=row, uops=uops, rd1_en=_has_src1(spec)
        ).sha(ver)
    op = dve_ops.DveOp(NAME, spec, subdim=False, uops_sha=shas)
    dve_ops.OPS.append(op)
    dve_ops._SUB_OPCODE_FOR_NAME[NAME] = row
    dve_ops.CUSTOM_DVE_SPECS[NAME] = spec
    _CACHE["op"] = op
    return op


def _build_program():
    from concourse import bacc, tile
    import concourse.mybir as mybir

    cumsum_op = _get_cumsum_op()

    f32 = mybir.dt.float32
    AF = mybir.ActivationFunctionType

    nc = bacc.Bacc("TRN2", debug=False, num_devices=N_CORES)
    xwd = nc.dram_tensor("xw", [128, NXW], f32, kind="ExternalInput")
    od = nc.dram_tensor("out", [1, PB], f32, kind="ExternalOutput")

    with tile.TileContext(nc) as tc:
        with (
            tc.tile_pool(name="sbuf", bufs=1) as pool,
            tc.tile_pool(name="psum", bufs=1, space="PSUM") as psum,
        ):
            xwt = pool.tile([128, NXW], f32)
            nc.sync.dma_start(out=xwt[:], in_=xwd[:])
            wt = xwt[:, 0:NW]
            xt = xwt[:, NW:NXW]  # partition=i, col=b*W+j
            xv = xt.rearrange("p (b j) -> p b j", b=PB)

            # C = cos(x) = sin(pi/2 - x); argument in (-pi/2, pi/2].
            # Two activations over shifted per-image windows write PACKED
            # (gap-free) tiles so the q-scan sees flat 2D operands (TTSS
            # encoding: both scan scalars may be runtime APs).
            NQ = PB * (W - 1)
            cta = pool.tile([128, NQ], f32)  # C[i, b, 0:127]
            nc.scalar.activation(
                out=cta[:].rearrange("p (b j) -> p b j", j=W - 1),
                in_=xv[:, :, 0 : W - 1],
                func=AF.Sin, bias=wt[:, 8:9], scale=-1.0,
            )
            ctb = pool.tile([128, NQ], f32)  # C[i, b, 1:128]
            nc.scalar.activation(
                out=ctb[:].rearrange("p (b j) -> p b j", j=W - 1),
                in_=xv[:, :, 1:W],
                func=AF.Sin, bias=wt[:, 8:9], scale=-1.0,
            )

            # xcum[i,k] = cumsum of x columns (body (x*0+1)*x = x)
            xcum = pool.tile([128, PB * W], f32)
            nc.vector._custom_dve(
                cumsum_op, out=xcum[:], in0=xt, in1=xt, s0=0.0, s1=1.0
            )
            xcv = xcum[:].rearrange("p (b j) -> p b j", b=PB)

            # qcum[i,k] = cumsum of (B*C[i,j+1] + A)*C[i,j], 127 per image
            qcum = pool.tile([128, NQ], f32)
            nc.vector._custom_dve(
                cumsum_op,
                out=qcum[:],
                in0=ctb[:],
                in1=cta[:],
                s0=wt[:, 6:7],
                s1=wt[:, 7:8],
            )
            qcv = qcum[:].rearrange("p (b j) -> p b j", b=PB)

            # six accumulating K=128 matmuls: per-image totals = weighted
            # partition sums of cumsum-sample differences + boundary columns
            pt = psum.tile([1, PB], f32)
            mm = nc.tensor.matmul
            kw = dict(skip_group_check=True)
            mm(pt[:, 0:PB], wt[:, 0:1], xcv[:, :, W - 1], start=True, stop=False, **kw)
            mm(pt[:, 1:PB], wt[:, 1:2], xcv[:, 0 : PB - 1, W - 1], start=False, stop=False, **kw)
            mm(pt[:, 0:PB], wt[:, 2:3], xv[:, :, 0], start=False, stop=False, **kw)
            mm(pt[:, 0:PB], wt[:, 3:4], xv[:, :, W - 1], start=False, stop=False, **kw)
            mm(pt[:, 0:PB], wt[:, 4:5], qcv[:, :, W - 2], start=False, stop=False, **kw)
            mm(pt[:, 1:PB], wt[:, 5:6], qcv[:, 0 : PB - 1, W - 2], start=False, stop=True, **kw)

            ot = pool.tile([1, PB], f32)
            nc.scalar.copy(out=ot[:], in_=pt[:])
            nc.sync.dma_start(out=od[:], in_=ot[:])

    nc.compile()
    return nc


def _weights(conv_w, conv_b, ry_angles, head_w, head_b):
    ry = np.asarray(ry_angles, np.float64)
    A = np.cos(ry[1, 0]) * np.cos(ry[0, 0])
    Bq = -np.sin(ry[1, 0]) * np.sin(ry[0, 0]) * np.sin(ry[0, 1])
    cw = np.asarray(conv_w, np.float64).reshape(4)
    hw = np.asarray(head_w, np.float64)
    h0, h1 = hw[0, 0], hw[0, 1]
    i = np.arange(128)
    m = (i < H - 1).astype(np.float64)
    mp = (i > 0).astype(np.float64)
    aa = h0 * (cw[0] * m + cw[2] * mp)
    ab = h0 * (cw[1] * m + cw[3] * mp)
    wt = np.zeros((128, NW), np.float32)
    wt[:, 0] = aa + ab
    wt[:, 1] = -(aa + ab)
    wt[:, 2] = -ab
    wt[:, 3] = -aa
    wt[:, 4] = h1 * m
    wt[:, 5] = -(h1 * m)
    wt[:, 6] = Bq
    wt[:, 7] = A
    wt[:, 8] = np.pi / 2  # bias column for cos(x) = sin(-x + pi/2)
    const = h0 * float(np.asarray(conv_b).reshape(-1)[0]) + float(
        np.asarray(head_b).reshape(-1)[0]
    )
    return wt, const


def run(x, conv_w, conv_b, ry_angles, head_w, head_b, trace=False, **run_kwargs):
    from concourse.bass_utils import run_bass_kernel_spmd

    if "nc" not in _CACHE:
        _CACHE["nc"] = _build_program()
    nc = _CACHE["nc"]

    wt, const = _weights(conv_w, conv_b, ry_angles, head_w, head_b)
    xs = np.asarray(x, np.float32).reshape(N_CORES, PB, H, W)
    # per-core packed input: [weight cols | x rows-major-by-partition]
    xperm = xs.transpose(0, 2, 1, 3).reshape(N_CORES, H, PB * W)
    xw = np.empty((N_CORES, H, NXW), np.float32)
    xw[:, :, :NW] = wt[None]
    xw[:, :, NW:] = xperm
    in_maps = [{"xw": xw[c]} for c in range(N_CORES)]
    res = run_bass_kernel_spmd(
        nc, in_maps, list(range(N_CORES)), trace=trace, **run_kwargs
    )
    T = np.concatenate(
        [np.asarray(res.results[c]["out"], np.float64).reshape(PB) for c in range(N_CORES)]
    )
    out = (T / NPATCH + const).astype(np.float32)[:, None]
    return out, res


def kernel(**inputs):
    out, _ = run(**inputs)
    return out
